# revision 8
# baseline (speedup 1.0000x reference)
"""Trainium2 Bass kernel for ConvNetWithGlobalPooling (batch-parallel grouped CNN).

Per-sample network: 3x(3x3 SAME conv + per-sample bias + relu) ->
global mean pool -> per-sample outer product with fc vector + bias.

Sharding: pure data parallel, 4 samples per core across 8 cores.

Device strategy (per sample):
  - Activations live in SBUF in "padded flat" layout: a [C, 66*66] image with
    zero borders, stored in a [C, 4358] buffer with one slack element on each
    end (image occupies buffer[1 : 4357]).
  - A 3x3 SAME conv output at padded-flat position i is
        sum_s W[s] @ x_pad[i + delta_s],  delta_s = (dy-1)*66 + (dx-1)
    so each shift's rhs is a *contiguous* slice of the padded buffer and the
    9 shifts accumulate into one PSUM bank (K-tiled im2col without any data
    movement). Output columns x=0,65 of each row are garbage; the PSUM->SBUF
    relu+bias copy uses a strided AP that skips them, so borders stay zero.
  - conv1 (Cin=3) uses a host-built im2col (27 rows = 9 shifts x 3 chans) so
    it is a single K=27 matmul per chunk instead of 9 K=3 matmuls.
  - Output rows are processed in chunks of 7 rows (N=462 <= 512 PSUM bank).
  - Matmul operands are bitcast to float32r (full PE rate at N>=256, ~tf32
    precision, fp32 PSUM accumulation).
  - conv3's relu copy also emits accum_out (per-partition running sum) ->
    global mean pooling is free; the 1/4096 is folded into the fc weight on
    the host. The final outer product + bias is two small DVE ops per half
    using host-replicated fc/bias rows.
"""

import os
import sys

sys.path.insert(0, "/opt/trn_rl_repo")

import numpy as np

import concourse.bass as bass
import concourse.bacc as bacc
import concourse.tile as tile
from concourse import mybir
from concourse.bass_utils import run_bass_kernel_spmd

F32 = mybir.dt.float32
F32R = mybir.dt.float32r
RELU = mybir.ActivationFunctionType.Relu

B = 32
N_CORES = 8
SPC = B // N_CORES  # samples per core
H = W = 64
PW = W + 2  # 66
PH = H + 2
NPIX = PH * PW  # 4356
PADLEN = NPIX + 2  # slack element at each end; image at [1, 4357)
ROWS_PER_CHUNK = 7
# (flat_start, nrows) for output rows 1..64 in padded coords
CHUNKS = []
_r = 1
while _r <= H:
    nr = min(ROWS_PER_CHUNK, H + 1 - _r)
    CHUNKS.append((_r * PW, nr))
    _r += nr
DELTAS = [(dy - 1) * PW + (dx - 1) for dy in range(3) for dx in range(3)]


def _conv_layer(nc, psum, src_pad, dst_pad, wts, bias_ap, cout, cin, name):
    """One 3x3 conv layer: 9-shift PSUM accumulation + relu/bias copy.

    src_pad: [cin, PADLEN] padded input. dst_pad: [cout, PADLEN] (pre-zeroed).
    wts: [cin, 9*cout_total] lhsT slices per shift (cout cols starting at off).
    """
    n_half = cout // 128 if cout > 128 else 1
    for h in range(n_half):
        for start, nrows in CHUNKS:
            size = nrows * PW
            p = min(cout, 128)
            ps = psum.tile([p, size], F32, name=f"ps_{name}", tag=f"ps_{name}")
            for i, d in enumerate(DELTAS):
                lhsT = wts[:, i * cout + h * 128 : i * cout + h * 128 + p]
                rhs = src_pad[:, 1 + start + d : 1 + start + d + size]
                nc.tensor.matmul(
                    ps[:],
                    lhsT,
                    rhs,
                    start=(i == 0),
                    stop=(i == len(DELTAS) - 1),
                )
            src = ps[:].rearrange("p (r c) -> p r c", c=PW)[:, :, 1:65]
            dst_region = dst_pad[h * 128 : h * 128 + p, 2 + start : 2 + start + size]
            dst = dst_region.rearrange("p (r c) -> p r c", c=PW)[:, :, 0:64]
            nc.scalar.activation(dst, src, RELU, bias=bias_ap[h])
    return


def build_nc():
    nc = bacc.Bacc(
        "TRN2",
        target_bir_lowering=False,
        debug=False,
        num_devices=N_CORES,
    )
    xim_d = nc.declare_dram_parameter("xim", [SPC, 27, NPIX], F32R, isOutput=False)
    w1_d = nc.declare_dram_parameter("w1", [SPC, 27, 64], F32R, isOutput=False)
    w2_d = nc.declare_dram_parameter("w2", [SPC, 64, 9 * 128], F32R, isOutput=False)
    w3_d = nc.declare_dram_parameter("w3", [SPC, 128, 9 * 256], F32R, isOutput=False)
    b1_d = nc.declare_dram_parameter("b1", [SPC, 64, 1], F32, isOutput=False)
    b2_d = nc.declare_dram_parameter("b2", [SPC, 128, 1], F32, isOutput=False)
    b3_d = nc.declare_dram_parameter("b3", [SPC, 128, 2], F32, isOutput=False)
    fcb_d = nc.declare_dram_parameter("fcb", [SPC, 128, 20], F32, isOutput=False)
    out_d = nc.declare_dram_parameter("out", [SPC, 128, 20], F32, isOutput=True)

    with tile.TileContext(nc) as tc:
        with (
            tc.tile_pool(name="wpool", bufs=2) as wpool,
            tc.tile_pool(name="apool", bufs=2) as apool,
            tc.tile_pool(name="spool", bufs=2) as spool,
            tc.tile_pool(name="scrpool", bufs=3) as scrpool,
            tc.tile_pool(name="psum", bufs=2, space="PSUM") as psum,
        ):
            for s in range(SPC):
                xim = apool.tile([27, NPIX], F32R)
                nc.sync.dma_start(xim[:], xim_d[s])
                w1 = wpool.tile([27, 64], F32R)
                nc.sync.dma_start(w1[:], w1_d[s])
                w2 = wpool.tile([64, 9 * 128], F32R)
                nc.sync.dma_start(w2[:], w2_d[s])
                w3 = wpool.tile([128, 9 * 256], F32R)
                nc.sync.dma_start(w3[:], w3_d[s])
                b1 = spool.tile([64, 1], F32)
                nc.sync.dma_start(b1[:], b1_d[s])
                b2 = spool.tile([128, 1], F32)
                nc.sync.dma_start(b2[:], b2_d[s])
                b3 = spool.tile([128, 2], F32)
                nc.sync.dma_start(b3[:], b3_d[s])
                fcb = spool.tile([128, 20], F32)
                nc.sync.dma_start(fcb[:], fcb_d[s])

                pad1 = apool.tile([64, PADLEN], F32R)
                nc.vector.memset(pad1[:].bitcast(F32), 0.0)
                pad2 = apool.tile([128, PADLEN], F32R)
                nc.vector.memset(pad2[:].bitcast(F32), 0.0)

                # conv1: single K=27 matmul per chunk from host im2col
                for start, nrows in CHUNKS:
                    size = nrows * PW
                    ps1 = psum.tile([64, size], F32, name="ps1", tag="ps1")
                    nc.tensor.matmul(
                        ps1[:],
                        w1[:],
                        xim[:, start : start + size],
                        start=True,
                        stop=True,
                    )
                    src = ps1[:].rearrange("p (r c) -> p r c", c=PW)[:, :, 1:65]
                    dreg = pad1[:, 2 + start : 2 + start + size]
                    dst = dreg.rearrange("p (r c) -> p r c", c=PW)[:, :, 0:64]
                    nc.scalar.activation(dst, src, RELU, bias=b1[:, 0:1])

                # conv2: 9-shift accumulation, Cout=128
                _conv_layer(
                    nc, psum, pad1, pad2, w2, [b2[:, 0:1]], 128, 64, "c2"
                )

                # conv3: 9-shift accumulation, Cout=256 (2 halves), relu copy
                # into scratch with accum_out -> pooled sums
                acc = spool.tile([128, 20], F32)
                for h in range(2):
                    for k, (start, nrows) in enumerate(CHUNKS):
                        size = nrows * PW
                        ps3 = psum.tile([128, size], F32, name="ps3", tag="ps3")
                        for i, d in enumerate(DELTAS):
                            lhsT = w3[:, i * 256 + h * 128 : i * 256 + h * 128 + 128]
                            rhs = pad2[:, 1 + start + d : 1 + start + d + size]
                            nc.tensor.matmul(
                                ps3[:],
                                lhsT,
                                rhs,
                                start=(i == 0),
                                stop=(i == 8),
                            )
                        src = ps3[:].rearrange("p (r c) -> p r c", c=PW)[:, :, 1:65]
                        scr = scrpool.tile([128, nrows * 64], F32, name="scr", tag="scr")
                        dst = scr[:].rearrange("p (r c) -> p r c", c=64)
                        idx = h * 10 + k
                        nc.scalar.activation(
                            dst,
                            src,
                            RELU,
                            bias=b3[:, h : h + 1],
                            accum_out=acc[:, idx : idx + 1],
                        )

                pooled = spool.tile([128, 2], F32)
                nc.vector.tensor_reduce(
                    pooled[:],
                    acc[:].rearrange("p (h o) -> p h o", h=2),
                    axis=mybir.AxisListType.X,
                    op=mybir.AluOpType.add,
                )

                outsb = spool.tile([128, 20], F32)
                for h in range(2):
                    tmp = spool.tile([128, 10], F32, name="tmp", tag="tmp")
                    nc.vector.tensor_scalar_mul(
                        tmp[:], fcb[:, 0:10], pooled[:, h : h + 1]
                    )
                    nc.vector.tensor_add(
                        outsb[:, h * 10 : h * 10 + 10], tmp[:], fcb[:, 10:20]
                    )
                nc.sync.dma_start(out_d[s], outsb[:])
    nc.compile()
    return nc


def round_f32r(a):
    """Round fp32 to the f32r (tf32-style) grid: RNE, drop low 13 mantissa bits."""
    u = np.ascontiguousarray(a, np.float32).view(np.uint32)
    bias = np.uint32(0xFFF) + ((u >> np.uint32(13)) & np.uint32(1))
    r = (u + bias) & np.uint32(0xFFFFE000)
    return r.view(np.float32)


def prep_inputs(x, conv1_weight, conv2_weight, conv3_weight, fc_weight,
                bias1, bias2, bias3, bias4):
    """Host-side layout prep (pure data movement, no model math)."""
    f = np.float32
    x = np.asarray(x, f)
    padx = np.zeros((B, 3, PH, PW), f)
    padx[:, :, 1:65, 1:65] = x
    padflat = padx.reshape(B, 3, NPIX)
    xim = np.zeros((B, 27, NPIX), f)
    for s, d in enumerate(DELTAS):
        lo = max(0, -d)
        hi = min(NPIX, NPIX - d)
        xim[:, s * 3 : s * 3 + 3, lo:hi] = padflat[:, :, lo + d : hi + d]

    w1 = np.ascontiguousarray(
        np.asarray(conv1_weight, f).transpose(0, 3, 4, 2, 1).reshape(B, 27, 64)
    )
    w2 = np.ascontiguousarray(
        np.asarray(conv2_weight, f).transpose(0, 2, 3, 4, 1).reshape(B, 64, 9 * 128)
    )
    w3 = np.ascontiguousarray(
        np.asarray(conv3_weight, f).transpose(0, 2, 3, 4, 1).reshape(B, 128, 9 * 256)
    )
    b1 = np.ascontiguousarray(np.asarray(bias1, f)[:, :, None])
    b2 = np.ascontiguousarray(np.asarray(bias2, f)[:, :, None])
    b3 = np.ascontiguousarray(np.asarray(bias3, f).reshape(B, 2, 128).transpose(0, 2, 1))
    fcs = np.asarray(fc_weight, f)[:, 0, :] / np.float32(H * W)
    fcb = np.concatenate(
        [
            np.repeat(fcs[:, None, :], 128, axis=1),
            np.repeat(np.asarray(bias4, f)[:, None, :], 128, axis=1),
        ],
        axis=2,
    )
    fcb = np.ascontiguousarray(fcb)
    # matmul operands are declared float32r on device; pre-round to the f32r
    # grid so host data matches what the PE computes with
    xim = round_f32r(xim)
    w1 = round_f32r(w1)
    w2 = round_f32r(w2)
    w3 = round_f32r(w3)
    return xim, w1, w2, w3, b1, b2, b3, fcb


_NC_CACHE = {}
LAST_RESULTS = None


def kernel(x, conv1_weight, conv2_weight, conv3_weight, fc_weight,
           bias1, bias2, bias3, bias4):
    global LAST_RESULTS
    xim, w1, w2, w3, b1, b2, b3, fcb = prep_inputs(
        x, conv1_weight, conv2_weight, conv3_weight, fc_weight,
        bias1, bias2, bias3, bias4,
    )
    if "nc" not in _NC_CACHE:
        _NC_CACHE["nc"] = build_nc()
    nc = _NC_CACHE["nc"]

    in_maps = []
    for c in range(N_CORES):
        sl = slice(c * SPC, (c + 1) * SPC)
        in_maps.append(
            {
                "xim": np.ascontiguousarray(xim[sl]),
                "w1": w1[sl],
                "w2": w2[sl],
                "w3": w3[sl],
                "b1": b1[sl],
                "b2": b2[sl],
                "b3": b3[sl],
                "fcb": fcb[sl],
            }
        )
    res = run_bass_kernel_spmd(nc, in_maps, list(range(N_CORES)))
    LAST_RESULTS = res
    outs = []
    for c in range(N_CORES):
        o = np.asarray(res.results[c]["out"], np.float32)  # [SPC, 128, 20]
        outs.append(o.reshape(SPC, 128, 2, 10).transpose(0, 2, 1, 3).reshape(SPC, 256, 10))
    return np.concatenate(outs, axis=0)


# revision 9
# speedup vs baseline: 1.1476x; 1.1476x over previous
"""Trainium2 Bass kernel for ConvNetWithGlobalPooling (batch-parallel grouped CNN).

Per-sample network: 3x(3x3 SAME conv + per-sample bias + relu) ->
global mean pool -> per-sample outer product with fc vector + bias.

Sharding: pure data parallel, 4 samples per core across 8 cores.

Device strategy (per sample):
  - Activations live in SBUF in "padded flat" layout: a [C, 66*66] image with
    zero borders, stored in a [C, 4358] buffer with one slack element on each
    end (image occupies buffer[1 : 4357]).
  - A 3x3 SAME conv output at padded-flat position i is
        sum_s W[s] @ x_pad[i + delta_s],  delta_s = (dy-1)*66 + (dx-1)
    so each shift's rhs is a *contiguous* slice of the padded buffer and the
    9 shifts accumulate into one PSUM bank (K-tiled im2col without any data
    movement). Output columns x=0,65 of each row are garbage; the PSUM->SBUF
    relu+bias copy uses a strided AP that skips them, so borders stay zero.
  - conv1 (Cin=3) uses a host-built im2col (27 rows = 9 shifts x 3 chans) so
    it is a single K=27 matmul per chunk instead of 9 K=3 matmuls.
  - Output rows are processed in chunks of 7 rows (N=462 <= 512 PSUM bank).
  - Matmul operands are bitcast to float32r (full PE rate at N>=256, ~tf32
    precision, fp32 PSUM accumulation).
  - conv3's relu copy also emits accum_out (per-partition running sum) ->
    global mean pooling is free; the 1/4096 is folded into the fc weight on
    the host. The final outer product + bias is two small DVE ops per half
    using host-replicated fc/bias rows.
"""

import os
import sys

sys.path.insert(0, "/opt/trn_rl_repo")

import numpy as np

import concourse.bass as bass
import concourse.bacc as bacc
import concourse.tile as tile
from concourse import mybir
from concourse.bass_utils import run_bass_kernel_spmd

F32 = mybir.dt.float32
F32R = mybir.dt.float32r
BF16 = mybir.dt.bfloat16
RELU = mybir.ActivationFunctionType.Relu

B = 32
N_CORES = 8
SPC = B // N_CORES  # samples per core
H = W = 64
PW = W + 2  # 66
PH = H + 2
NPIX = PH * PW  # 4356
PADLEN = NPIX + 2  # slack element at each end; image at [1, 4357)
ROWS_PER_CHUNK = 7
# (flat_start, nrows) for output rows 1..64 in padded coords
CHUNKS = []
_r = 1
while _r <= H:
    nr = min(ROWS_PER_CHUNK, H + 1 - _r)
    CHUNKS.append((_r * PW, nr))
    _r += nr
DELTAS = [(dy - 1) * PW + (dx - 1) for dy in range(3) for dx in range(3)]


def _conv_layer(nc, psum, src_pad, dst_pad, wts, bias_ap, cout, cin, name):
    """One 3x3 conv layer: 9-shift PSUM accumulation + relu/bias copy.

    src_pad: [cin, PADLEN] padded input. dst_pad: [cout, PADLEN] (pre-zeroed).
    wts: [cin, 9*cout_total] lhsT slices per shift (cout cols starting at off).
    """
    n_half = cout // 128 if cout > 128 else 1
    for h in range(n_half):
        for start, nrows in CHUNKS:
            size = nrows * PW
            p = min(cout, 128)
            ps = psum.tile([p, size], F32, name=f"ps_{name}", tag=f"ps_{name}")
            for i, d in enumerate(DELTAS):
                lhsT = wts[:, i * cout + h * 128 : i * cout + h * 128 + p]
                rhs = src_pad[:, 1 + start + d : 1 + start + d + size]
                nc.tensor.matmul(
                    ps[:],
                    lhsT,
                    rhs,
                    start=(i == 0),
                    stop=(i == len(DELTAS) - 1),
                )
            src = ps[:].rearrange("p (r c) -> p r c", c=PW)[:, :, 1:65]
            dst_region = dst_pad[h * 128 : h * 128 + p, 2 + start : 2 + start + size]
            dst = dst_region.rearrange("p (r c) -> p r c", c=PW)[:, :, 0:64]
            nc.scalar.activation(dst, src, RELU, bias=bias_ap[h])
    return


def build_nc():
    nc = bacc.Bacc(
        "TRN2",
        target_bir_lowering=False,
        debug=False,
        num_devices=N_CORES,
    )
    xim_d = nc.declare_dram_parameter("xim", [SPC, 27, NPIX], BF16, isOutput=False)
    w1_d = nc.declare_dram_parameter("w1", [SPC, 27, 64], BF16, isOutput=False)
    w2_d = nc.declare_dram_parameter("w2", [SPC, 64, 9 * 128], BF16, isOutput=False)
    w3_d = nc.declare_dram_parameter("w3", [SPC, 128, 9 * 256], BF16, isOutput=False)
    b1_d = nc.declare_dram_parameter("b1", [SPC, 64, 1], F32, isOutput=False)
    b2_d = nc.declare_dram_parameter("b2", [SPC, 128, 1], F32, isOutput=False)
    b3_d = nc.declare_dram_parameter("b3", [SPC, 128, 2], F32, isOutput=False)
    fcb_d = nc.declare_dram_parameter("fcb", [SPC, 128, 20], F32, isOutput=False)
    out_d = nc.declare_dram_parameter("out", [SPC, 128, 20], F32, isOutput=True)

    with tile.TileContext(nc) as tc:
        with (
            tc.tile_pool(name="wpool", bufs=2) as wpool,
            tc.tile_pool(name="apool", bufs=2) as apool,
            tc.tile_pool(name="spool", bufs=2) as spool,
            tc.tile_pool(name="scrpool", bufs=3) as scrpool,
            tc.tile_pool(name="psum", bufs=2, space="PSUM") as psum,
        ):
            for s in range(SPC):
                xim = apool.tile([27, NPIX], BF16)
                nc.sync.dma_start(xim[:], xim_d[s])
                w1 = wpool.tile([27, 64], BF16)
                nc.sync.dma_start(w1[:], w1_d[s])
                w2 = wpool.tile([64, 9 * 128], BF16)
                nc.sync.dma_start(w2[:], w2_d[s])
                w3 = wpool.tile([128, 9 * 256], BF16)
                nc.sync.dma_start(w3[:], w3_d[s])
                b1 = spool.tile([64, 1], F32)
                nc.sync.dma_start(b1[:], b1_d[s])
                b2 = spool.tile([128, 1], F32)
                nc.sync.dma_start(b2[:], b2_d[s])
                b3 = spool.tile([128, 2], F32)
                nc.sync.dma_start(b3[:], b3_d[s])
                fcb = spool.tile([128, 20], F32)
                nc.sync.dma_start(fcb[:], fcb_d[s])

                pad1 = apool.tile([64, PADLEN], BF16)
                nc.vector.memset(pad1[:], 0.0)
                pad2 = apool.tile([128, PADLEN], BF16)
                nc.vector.memset(pad2[:], 0.0)

                # conv1: single K=27 matmul per chunk from host im2col
                for start, nrows in CHUNKS:
                    size = nrows * PW
                    ps1 = psum.tile([64, size], F32, name="ps1", tag="ps1")
                    nc.tensor.matmul(
                        ps1[:],
                        w1[:],
                        xim[:, start : start + size],
                        start=True,
                        stop=True,
                    )
                    src = ps1[:].rearrange("p (r c) -> p r c", c=PW)[:, :, 1:65]
                    dreg = pad1[:, 2 + start : 2 + start + size]
                    dst = dreg.rearrange("p (r c) -> p r c", c=PW)[:, :, 0:64]
                    nc.scalar.activation(dst, src, RELU, bias=b1[:, 0:1])

                # conv2: 9-shift accumulation, Cout=128
                _conv_layer(
                    nc, psum, pad1, pad2, w2, [b2[:, 0:1]], 128, 64, "c2"
                )

                # conv3: 9-shift accumulation, Cout=256 (2 halves), relu copy
                # into scratch with accum_out -> pooled sums
                acc = spool.tile([128, 20], F32)
                for h in range(2):
                    for k, (start, nrows) in enumerate(CHUNKS):
                        size = nrows * PW
                        ps3 = psum.tile([128, size], F32, name="ps3", tag="ps3")
                        for i, d in enumerate(DELTAS):
                            lhsT = w3[:, i * 256 + h * 128 : i * 256 + h * 128 + 128]
                            rhs = pad2[:, 1 + start + d : 1 + start + d + size]
                            nc.tensor.matmul(
                                ps3[:],
                                lhsT,
                                rhs,
                                start=(i == 0),
                                stop=(i == 8),
                            )
                        src = ps3[:].rearrange("p (r c) -> p r c", c=PW)[:, :, 1:65]
                        scr = scrpool.tile([128, nrows * 64], F32, name="scr", tag="scr")
                        dst = scr[:].rearrange("p (r c) -> p r c", c=64)
                        idx = h * 10 + k
                        nc.scalar.activation(
                            dst,
                            src,
                            RELU,
                            bias=b3[:, h : h + 1],
                            accum_out=acc[:, idx : idx + 1],
                        )

                pooled = spool.tile([128, 2], F32)
                nc.vector.tensor_reduce(
                    pooled[:],
                    acc[:].rearrange("p (h o) -> p h o", h=2),
                    axis=mybir.AxisListType.X,
                    op=mybir.AluOpType.add,
                )

                outsb = spool.tile([128, 20], F32)
                for h in range(2):
                    tmp = spool.tile([128, 10], F32, name="tmp", tag="tmp")
                    nc.vector.tensor_scalar_mul(
                        tmp[:], fcb[:, 0:10], pooled[:, h : h + 1]
                    )
                    nc.vector.tensor_add(
                        outsb[:, h * 10 : h * 10 + 10], tmp[:], fcb[:, 10:20]
                    )
                nc.sync.dma_start(out_d[s], outsb[:])
    nc.compile()
    return nc


def round_f32r(a):
    """Round fp32 to the f32r (tf32-style) grid: RNE, drop low 13 mantissa bits."""
    u = np.ascontiguousarray(a, np.float32).view(np.uint32)
    bias = np.uint32(0xFFF) + ((u >> np.uint32(13)) & np.uint32(1))
    r = (u + bias) & np.uint32(0xFFFFE000)
    return r.view(np.float32)


def prep_inputs(x, conv1_weight, conv2_weight, conv3_weight, fc_weight,
                bias1, bias2, bias3, bias4):
    """Host-side layout prep (pure data movement, no model math)."""
    f = np.float32
    x = np.asarray(x, f)
    padx = np.zeros((B, 3, PH, PW), f)
    padx[:, :, 1:65, 1:65] = x
    padflat = padx.reshape(B, 3, NPIX)
    xim = np.zeros((B, 27, NPIX), f)
    for s, d in enumerate(DELTAS):
        lo = max(0, -d)
        hi = min(NPIX, NPIX - d)
        xim[:, s * 3 : s * 3 + 3, lo:hi] = padflat[:, :, lo + d : hi + d]

    w1 = np.ascontiguousarray(
        np.asarray(conv1_weight, f).transpose(0, 3, 4, 2, 1).reshape(B, 27, 64)
    )
    w2 = np.ascontiguousarray(
        np.asarray(conv2_weight, f).transpose(0, 2, 3, 4, 1).reshape(B, 64, 9 * 128)
    )
    w3 = np.ascontiguousarray(
        np.asarray(conv3_weight, f).transpose(0, 2, 3, 4, 1).reshape(B, 128, 9 * 256)
    )
    b1 = np.ascontiguousarray(np.asarray(bias1, f)[:, :, None])
    b2 = np.ascontiguousarray(np.asarray(bias2, f)[:, :, None])
    b3 = np.ascontiguousarray(np.asarray(bias3, f).reshape(B, 2, 128).transpose(0, 2, 1))
    fcs = np.asarray(fc_weight, f)[:, 0, :] / np.float32(H * W)
    fcb = np.concatenate(
        [
            np.repeat(fcs[:, None, :], 128, axis=1),
            np.repeat(np.asarray(bias4, f)[:, None, :], 128, axis=1),
        ],
        axis=2,
    )
    fcb = np.ascontiguousarray(fcb)
    # matmul operands are bf16 on device (1 cyc/row on the PE vs 2 for
    # fp32/fp32r); convert with RNE here
    import ml_dtypes
    bf = ml_dtypes.bfloat16
    xim = xim.astype(bf)
    w1 = w1.astype(bf)
    w2 = w2.astype(bf)
    w3 = w3.astype(bf)
    return xim, w1, w2, w3, b1, b2, b3, fcb


_NC_CACHE = {}
LAST_RESULTS = None


def kernel(x, conv1_weight, conv2_weight, conv3_weight, fc_weight,
           bias1, bias2, bias3, bias4):
    global LAST_RESULTS
    xim, w1, w2, w3, b1, b2, b3, fcb = prep_inputs(
        x, conv1_weight, conv2_weight, conv3_weight, fc_weight,
        bias1, bias2, bias3, bias4,
    )
    if "nc" not in _NC_CACHE:
        _NC_CACHE["nc"] = build_nc()
    nc = _NC_CACHE["nc"]

    in_maps = []
    for c in range(N_CORES):
        sl = slice(c * SPC, (c + 1) * SPC)
        in_maps.append(
            {
                "xim": np.ascontiguousarray(xim[sl]),
                "w1": w1[sl],
                "w2": w2[sl],
                "w3": w3[sl],
                "b1": b1[sl],
                "b2": b2[sl],
                "b3": b3[sl],
                "fcb": fcb[sl],
            }
        )
    res = run_bass_kernel_spmd(nc, in_maps, list(range(N_CORES)))
    LAST_RESULTS = res
    outs = []
    for c in range(N_CORES):
        o = np.asarray(res.results[c]["out"], np.float32)  # [SPC, 128, 20]
        outs.append(o.reshape(SPC, 128, 2, 10).transpose(0, 2, 1, 3).reshape(SPC, 256, 10))
    return np.concatenate(outs, axis=0)


# revision 10
# speedup vs baseline: 1.4268x; 1.2433x over previous
"""Trainium2 Bass kernel for ConvNetWithGlobalPooling (batch-parallel grouped CNN).

Per-sample network: 3x(3x3 SAME conv + per-sample bias + relu) ->
global mean pool -> per-sample outer product with fc vector + bias.

Sharding: pure data parallel, 4 samples per core across 8 cores.

Device strategy (per sample):
  - Activations live in SBUF in "padded flat" layout: a [C, 66*66] image with
    zero borders stored in a [C, 4360] buffer (image at buffer[1 : 4357]).
  - A 3x3 SAME conv output at padded-flat position i is
        sum_s W[s] @ x_pad[i + delta_s],  delta_s = (dy-1)*66 + (dx-1)
    so each shift's rhs is a strided view of the padded buffer and shifts
    accumulate into one PSUM bank (K-tiled im2col with zero data movement).
    Matmul rhs uses a 2-level AP ([rows x 66-stride][64 cols]) so only the 64
    valid columns of each row are streamed (no garbage columns computed).
  - conv1 (Cin=3) uses a host-built im2col (27 rows = 9 shifts x 3 chans):
    a single K=27 matmul per chunk.
  - conv2 shift-pairing: conv1's relu-copy writes its output TWICE into a
    [128, 4360] buffer - rows 0:64 normal, rows 64:128 shifted down one image
    row (+66). A K=128 matmul against stacked weights then computes TWO
    shifts at once: dy=0 paired with dy=1 (3 matmuls), dy=2 alone in the top
    half with zero bottom weights (3 matmuls) -> 6 streams instead of 9.
  - Output rows are processed in chunks of 7 rows (N=448 <= 512 PSUM bank).
  - All matmul operands are bf16 (1 PE cycle/row vs 2 for fp32/fp32r),
    fp32 PSUM accumulation.
  - conv3's relu copy emits accum_out (per-partition running sum) -> global
    mean pooling is free; 1/4096 is folded into the fc weight on the host.
    The final outer product + bias is two small DVE ops per half using
    host-replicated fc/bias rows.
"""

import os
import sys

sys.path.insert(0, "/opt/trn_rl_repo")

import numpy as np

import concourse.bass as bass
import concourse.bacc as bacc
import concourse.tile as tile
from concourse import mybir
from concourse.bass_utils import run_bass_kernel_spmd

F32 = mybir.dt.float32
BF16 = mybir.dt.bfloat16
RELU = mybir.ActivationFunctionType.Relu

B = 32
N_CORES = 8
SPC = B // N_CORES  # samples per core
H = W = 64
PW = W + 2  # 66
PH = H + 2
NPIX = PH * PW  # 4356
PADLEN = NPIX + 4  # 1 slack front + 3 slack tail; image at [1, 4357)
ROWS_PER_CHUNK = 7
# (first_output_row r0 in 1..64, nrows) chunks
RCHUNKS = []
_r = 1
while _r <= H:
    nr = min(ROWS_PER_CHUNK, H + 1 - _r)
    RCHUNKS.append((_r, nr))
    _r += nr
DELTAS = [(dy - 1) * PW + (dx - 1) for dy in range(3) for dx in range(3)]
# conv2 pairing: matmul j<3 pairs (dy=0,dx=j) with (dy=1,dx=j) via the
# row-shifted bottom half; j=3..5 is (dy=2,dx=j-3) in the top half only.
C2_DELTAS = [DELTAS[j] for j in range(3)] + [DELTAS[6 + j] for j in range(3)]


def _rview(ap, off, nrows):
    """[P, nrows, 64] view of a padded-flat buffer starting at `off`."""
    return ap[:, off : off + nrows * PW].rearrange("p (r c) -> p r c", c=PW)[
        :, :, 0:64
    ]


def build_nc():
    nc = bacc.Bacc(
        "TRN2",
        target_bir_lowering=False,
        debug=False,
        num_devices=N_CORES,
    )
    xim_d = nc.declare_dram_parameter("xim", [SPC, 27, NPIX], BF16, isOutput=False)
    w1_d = nc.declare_dram_parameter("w1", [SPC, 27, 64], BF16, isOutput=False)
    w2_d = nc.declare_dram_parameter("w2", [SPC, 128, 6 * 128], BF16, isOutput=False)
    w3_d = nc.declare_dram_parameter("w3", [SPC, 128, 9 * 256], BF16, isOutput=False)
    b1_d = nc.declare_dram_parameter("b1", [SPC, 64, 1], F32, isOutput=False)
    b2_d = nc.declare_dram_parameter("b2", [SPC, 128, 1], F32, isOutput=False)
    b3_d = nc.declare_dram_parameter("b3", [SPC, 128, 2], F32, isOutput=False)
    fcb_d = nc.declare_dram_parameter("fcb", [SPC, 128, 20], F32, isOutput=False)
    out_d = nc.declare_dram_parameter("out", [SPC, 128, 20], F32, isOutput=True)

    with tile.TileContext(nc) as tc:
        with (
            tc.tile_pool(name="wpool", bufs=2) as wpool,
            tc.tile_pool(name="apool", bufs=2) as apool,
            tc.tile_pool(name="spool", bufs=2) as spool,
            tc.tile_pool(name="scrpool", bufs=3) as scrpool,
            tc.tile_pool(name="psum", bufs=2, space="PSUM") as psum,
        ):
            for s in range(SPC):
                xim = apool.tile([27, NPIX], BF16)
                nc.sync.dma_start(xim[:], xim_d[s])
                w1 = wpool.tile([27, 64], BF16)
                nc.sync.dma_start(w1[:], w1_d[s])
                w2 = wpool.tile([128, 6 * 128], BF16)
                nc.sync.dma_start(w2[:], w2_d[s])
                w3 = wpool.tile([128, 9 * 256], BF16)
                nc.sync.dma_start(w3[:], w3_d[s])
                b1 = spool.tile([64, 1], F32)
                nc.sync.dma_start(b1[:], b1_d[s])
                b2 = spool.tile([128, 1], F32)
                nc.sync.dma_start(b2[:], b2_d[s])
                b3 = spool.tile([128, 2], F32)
                nc.sync.dma_start(b3[:], b3_d[s])
                fcb = spool.tile([128, 20], F32)
                nc.sync.dma_start(fcb[:], fcb_d[s])

                # pad1: rows 0:64 = conv1 output (padded), rows 64:128 = same
                # image shifted down one row (content[i] = top[i+66])
                pad1 = apool.tile([128, PADLEN], BF16)
                nc.vector.memset(pad1[:], 0.0)
                pad2 = apool.tile([128, PADLEN], BF16)
                nc.vector.memset(pad2[:], 0.0)

                # conv1: single K=27 matmul per chunk from host im2col,
                # relu+bias copied into BOTH halves of pad1
                for r0, nrows in RCHUNKS:
                    n = nrows * 64
                    base = r0 * PW  # flat offset of first output row
                    ps1 = psum.tile([64, n], F32, name="ps1", tag="ps1")
                    rhs = _rview(xim, base + 1, nrows)
                    nc.tensor.matmul(ps1[:], w1[:], rhs, start=True, stop=True)
                    src = ps1[:].rearrange("p (r c) -> p r c", c=64)
                    dst_t = _rview(pad1[0:64, :], base + 2, nrows)
                    nc.scalar.activation(dst_t, src, RELU, bias=b1[:, 0:1])
                    dst_b = _rview(pad1[64:128, :], base + 2 - PW, nrows)
                    nc.scalar.activation(dst_b, src, RELU, bias=b1[:, 0:1])

                # conv2: 6 K=128 matmuls per chunk (3 dy-paired + 3 single)
                for r0, nrows in RCHUNKS:
                    n = nrows * 64
                    base = r0 * PW
                    ps2 = psum.tile([128, n], F32, name="ps2", tag="ps2")
                    for j, dd in enumerate(C2_DELTAS):
                        lhsT = w2[:, j * 128 : (j + 1) * 128]
                        rhs = _rview(pad1, base + 2 + dd, nrows)
                        nc.tensor.matmul(
                            ps2[:], lhsT, rhs,
                            start=(j == 0), stop=(j == len(C2_DELTAS) - 1),
                        )
                    src = ps2[:].rearrange("p (r c) -> p r c", c=64)
                    dst = _rview(pad2, base + 2, nrows)
                    nc.scalar.activation(dst, src, RELU, bias=b2[:, 0:1])

                # conv3: 9 shifts x 2 Cout halves; relu copy into scratch
                # with accum_out -> pooled sums
                acc = spool.tile([128, 20], F32)
                for h in range(2):
                    for k, (r0, nrows) in enumerate(RCHUNKS):
                        n = nrows * 64
                        base = r0 * PW
                        ps3 = psum.tile([128, n], F32, name="ps3", tag="ps3")
                        for i, dd in enumerate(DELTAS):
                            lhsT = w3[:, i * 256 + h * 128 : i * 256 + h * 128 + 128]
                            rhs = _rview(pad2, base + 2 + dd, nrows)
                            nc.tensor.matmul(
                                ps3[:], lhsT, rhs,
                                start=(i == 0), stop=(i == 8),
                            )
                        scr = scrpool.tile([128, n], F32, name="scr", tag="scr")
                        idx = h * 10 + k
                        nc.scalar.activation(
                            scr[:], ps3[:], RELU,
                            bias=b3[:, h : h + 1],
                            accum_out=acc[:, idx : idx + 1],
                        )

                pooled = spool.tile([128, 2], F32)
                nc.vector.tensor_reduce(
                    pooled[:],
                    acc[:].rearrange("p (h o) -> p h o", h=2),
                    axis=mybir.AxisListType.X,
                    op=mybir.AluOpType.add,
                )

                outsb = spool.tile([128, 20], F32)
                for h in range(2):
                    tmp = spool.tile([128, 10], F32, name="tmp", tag="tmp")
                    nc.vector.tensor_scalar_mul(
                        tmp[:], fcb[:, 0:10], pooled[:, h : h + 1]
                    )
                    nc.vector.tensor_add(
                        outsb[:, h * 10 : h * 10 + 10], tmp[:], fcb[:, 10:20]
                    )
                nc.sync.dma_start(out_d[s], outsb[:])
    nc.compile()
    return nc


def prep_inputs(x, conv1_weight, conv2_weight, conv3_weight, fc_weight,
                bias1, bias2, bias3, bias4):
    """Host-side layout prep (pure data movement, no model math)."""
    import ml_dtypes

    f = np.float32
    bf = ml_dtypes.bfloat16
    x = np.asarray(x, f)
    padx = np.zeros((B, 3, PH, PW), f)
    padx[:, :, 1:65, 1:65] = x
    padflat = padx.reshape(B, 3, NPIX)
    xim = np.zeros((B, 27, NPIX), f)
    for s, d in enumerate(DELTAS):
        lo = max(0, -d)
        hi = min(NPIX, NPIX - d)
        xim[:, s * 3 : s * 3 + 3, lo:hi] = padflat[:, :, lo + d : hi + d]

    w1 = np.ascontiguousarray(
        np.asarray(conv1_weight, f).transpose(0, 3, 4, 2, 1).reshape(B, 27, 64)
    )
    # conv2 stacked-pair weights: [b, 2*ci, 6, co]
    w2n = np.asarray(conv2_weight, f).transpose(0, 2, 3, 4, 1).reshape(B, 64, 9, 128)
    w2p = np.zeros((B, 128, 6, 128), f)
    for j in range(3):
        w2p[:, 0:64, j] = w2n[:, :, j]          # dy=0, dx=j  (top half)
        w2p[:, 64:128, j] = w2n[:, :, 3 + j]    # dy=1, dx=j  (shifted bottom)
        w2p[:, 0:64, 3 + j] = w2n[:, :, 6 + j]  # dy=2, dx=j  (top, bottom=0)
    w2 = np.ascontiguousarray(w2p.reshape(B, 128, 6 * 128))
    w3 = np.ascontiguousarray(
        np.asarray(conv3_weight, f).transpose(0, 2, 3, 4, 1).reshape(B, 128, 9 * 256)
    )
    b1 = np.ascontiguousarray(np.asarray(bias1, f)[:, :, None])
    b2 = np.ascontiguousarray(np.asarray(bias2, f)[:, :, None])
    b3 = np.ascontiguousarray(np.asarray(bias3, f).reshape(B, 2, 128).transpose(0, 2, 1))
    fcs = np.asarray(fc_weight, f)[:, 0, :] / np.float32(H * W)
    fcb = np.concatenate(
        [
            np.repeat(fcs[:, None, :], 128, axis=1),
            np.repeat(np.asarray(bias4, f)[:, None, :], 128, axis=1),
        ],
        axis=2,
    )
    fcb = np.ascontiguousarray(fcb)
    return (xim.astype(bf), w1.astype(bf), w2.astype(bf), w3.astype(bf),
            b1, b2, b3, fcb)


_NC_CACHE = {}
LAST_RESULTS = None


def kernel(x, conv1_weight, conv2_weight, conv3_weight, fc_weight,
           bias1, bias2, bias3, bias4):
    global LAST_RESULTS
    xim, w1, w2, w3, b1, b2, b3, fcb = prep_inputs(
        x, conv1_weight, conv2_weight, conv3_weight, fc_weight,
        bias1, bias2, bias3, bias4,
    )
    if "nc" not in _NC_CACHE:
        _NC_CACHE["nc"] = build_nc()
    nc = _NC_CACHE["nc"]

    in_maps = []
    for c in range(N_CORES):
        sl = slice(c * SPC, (c + 1) * SPC)
        in_maps.append(
            {
                "xim": np.ascontiguousarray(xim[sl]),
                "w1": np.ascontiguousarray(w1[sl]),
                "w2": np.ascontiguousarray(w2[sl]),
                "w3": np.ascontiguousarray(w3[sl]),
                "b1": np.ascontiguousarray(b1[sl]),
                "b2": np.ascontiguousarray(b2[sl]),
                "b3": np.ascontiguousarray(b3[sl]),
                "fcb": np.ascontiguousarray(fcb[sl]),
            }
        )
    res = run_bass_kernel_spmd(nc, in_maps, list(range(N_CORES)))
    LAST_RESULTS = res
    outs = []
    for c in range(N_CORES):
        o = np.asarray(res.results[c]["out"], np.float32)  # [SPC, 128, 20]
        outs.append(o.reshape(SPC, 128, 2, 10).transpose(0, 2, 1, 3).reshape(SPC, 256, 10))
    return np.concatenate(outs, axis=0)


# revision 11
# speedup vs baseline: 1.4928x; 1.0462x over previous
"""Trainium2 Bass kernel for ConvNetWithGlobalPooling (batch-parallel grouped CNN).

Per-sample network: 3x(3x3 SAME conv + per-sample bias + relu) ->
global mean pool -> per-sample outer product with fc vector + bias.

Sharding: pure data parallel, 4 samples per core across 8 cores.

Device strategy (per sample):
  - Activations live in SBUF in "padded flat" layout: a [C, 66*66] image with
    zero borders stored in a [C, 4360] buffer (image at buffer[1 : 4357]).
  - A 3x3 SAME conv output at padded-flat position i is
        sum_s W[s] @ x_pad[i + delta_s],  delta_s = (dy-1)*66 + (dx-1)
    so each shift's rhs is a strided view of the padded buffer and shifts
    accumulate into one PSUM bank (K-tiled im2col with zero data movement).
    Matmul rhs uses a 2-level AP ([rows x 66-stride][64 cols]) so only the 64
    valid columns of each row are streamed (no garbage columns computed).
  - conv1 (Cin=3) uses a host-built im2col (27 rows = 9 shifts x 3 chans):
    a single K=27 matmul per chunk.
  - conv2 shift-pairing: conv1's relu-drain writes its output TWICE into a
    [128, 4360] buffer - rows 0:64 normal, rows 64:128 shifted down one image
    row (+66). A K=128 matmul against stacked weights then computes TWO
    shifts at once: dy=0 paired with dy=1 (3 matmuls), dy=2 alone in the top
    half with zero bottom weights (3 matmuls) -> 6 streams instead of 9.
  - Output rows are processed in 8 chunks of 8 rows (N=512 = one PSUM bank).
  - All matmul operands are bf16 (1 PE cycle/row vs 2 for fp32/fp32r),
    fp32 PSUM accumulation.
  - Engine split: PE does matmuls; DVE does conv1/conv2 relu+bias drains
    (tensor_scalar add-bias/max-0) and the fc stage; ACT does conv3 drains
    because activation's accum_out gives the global-sum pooling for free
    (1/4096 folded into the fc weight on host).
  - Emission is software-pipelined: conv1 of sample s+1 is emitted between
    conv3's two Cout halves of sample s so the PE never starves at sample
    boundaries (which would also re-engage the HAM half-clock throttle).
"""

import os
import sys

sys.path.insert(0, "/opt/trn_rl_repo")

import numpy as np

import concourse.bass as bass
import concourse.bacc as bacc
import concourse.tile as tile
from concourse import mybir
from concourse.bass_utils import run_bass_kernel_spmd

F32 = mybir.dt.float32
BF16 = mybir.dt.bfloat16
RELU = mybir.ActivationFunctionType.Relu
ADD = mybir.AluOpType.add
MAX = mybir.AluOpType.max

B = 32
N_CORES = 8
SPC = B // N_CORES  # samples per core
H = W = 64
PW = W + 2  # 66
PH = H + 2
NPIX = PH * PW  # 4356
PADLEN = NPIX + 4  # 1 slack front + 3 slack tail; image at [1, 4357)
ROWS_PER_CHUNK = 8  # N = 8*64 = 512 = exactly one PSUM bank of fp32
RCHUNKS = [(1 + 8 * k, 8) for k in range(8)]
DELTAS = [(dy - 1) * PW + (dx - 1) for dy in range(3) for dx in range(3)]
# conv2 pairing: matmul j<3 pairs (dy=0,dx=j) with (dy=1,dx=j) via the
# row-shifted bottom half; j=3..5 is (dy=2,dx=j-3) in the top half only.
C2_DELTAS = [DELTAS[j] for j in range(3)] + [DELTAS[6 + j] for j in range(3)]


def _rview(ap, off, nrows):
    """[P, nrows, 64] view of a padded-flat buffer starting at `off`."""
    return ap[:, off : off + nrows * PW].rearrange("p (r c) -> p r c", c=PW)[
        :, :, 0:64
    ]


def build_nc():
    nc = bacc.Bacc(
        "TRN2",
        target_bir_lowering=False,
        debug=False,
        num_devices=N_CORES,
    )
    xim_d = nc.declare_dram_parameter("xim", [SPC, 27, NPIX], BF16, isOutput=False)
    w1_d = nc.declare_dram_parameter("w1", [SPC, 27, 64], BF16, isOutput=False)
    w2_d = nc.declare_dram_parameter("w2", [SPC, 128, 6 * 128], BF16, isOutput=False)
    w3_d = nc.declare_dram_parameter("w3", [SPC, 128, 9 * 256], BF16, isOutput=False)
    b1_d = nc.declare_dram_parameter("b1", [SPC, 64, 1], F32, isOutput=False)
    b2_d = nc.declare_dram_parameter("b2", [SPC, 128, 1], F32, isOutput=False)
    b3_d = nc.declare_dram_parameter("b3", [SPC, 128, 2], F32, isOutput=False)
    fcb_d = nc.declare_dram_parameter("fcb", [SPC, 128, 20], F32, isOutput=False)
    out_d = nc.declare_dram_parameter("out", [SPC, 128, 20], F32, isOutput=True)

    with tile.TileContext(nc) as tc:
        with (
            tc.tile_pool(name="wpool", bufs=2) as wpool,
            tc.tile_pool(name="apool", bufs=2) as apool,
            tc.tile_pool(name="spool", bufs=2) as spool,
            tc.tile_pool(name="scrpool", bufs=3) as scrpool,
            tc.tile_pool(name="psum", bufs=2, space="PSUM") as psum,
        ):
            T = [None] * SPC  # per-sample tile dict

            def emit_loads(s):
                t = {}
                t["w1"] = wpool.tile([27, 64], BF16, name="w1", tag="w1")
                nc.sync.dma_start(t["w1"][:], w1_d[s])
                t["b1"] = spool.tile([64, 1], F32, name="b1", tag="b1")
                nc.sync.dma_start(t["b1"][:], b1_d[s])
                t["xim"] = apool.tile([27, NPIX], BF16, name="xim", tag="xim")
                # chunk-aligned pieces so conv1's first matmul starts early
                for r0, nrows in RCHUNKS:
                    lo = r0 * PW + 1
                    ln = nrows * PW
                    nc.sync.dma_start(
                        t["xim"][:, lo : lo + ln], xim_d[s][:, lo : lo + ln]
                    )
                t["w2"] = wpool.tile([128, 6 * 128], BF16, name="w2", tag="w2")
                nc.sync.dma_start(t["w2"][:], w2_d[s])
                t["w3"] = wpool.tile([128, 9 * 256], BF16, name="w3", tag="w3")
                nc.sync.dma_start(t["w3"][:], w3_d[s])
                t["b2"] = spool.tile([128, 1], F32, name="b2", tag="b2")
                nc.sync.dma_start(t["b2"][:], b2_d[s])
                t["b3"] = spool.tile([128, 2], F32, name="b3", tag="b3")
                nc.sync.dma_start(t["b3"][:], b3_d[s])
                t["fcb"] = spool.tile([128, 20], F32, name="fcb", tag="fcb")
                nc.sync.dma_start(t["fcb"][:], fcb_d[s])
                # pad1: rows 0:64 = conv1 out (padded), rows 64:128 = same
                # image shifted down one row (content[i] = top[i+66])
                t["pad1"] = apool.tile([128, PADLEN], BF16, name="pad1", tag="pad1")
                nc.vector.memset(t["pad1"][:], 0.0)
                t["pad2"] = apool.tile([128, PADLEN], BF16, name="pad2", tag="pad2")
                nc.vector.memset(t["pad2"][:], 0.0)
                return t

            def emit_conv1(t):
                # K=27 matmul per chunk; relu+bias drains on DVE into both
                # halves of pad1
                for r0, nrows in RCHUNKS:
                    n = nrows * 64
                    base = r0 * PW
                    ps1 = psum.tile([64, n], F32, name="ps1", tag="ps1")
                    rhs = _rview(t["xim"], base + 1, nrows)
                    nc.tensor.matmul(ps1[:], t["w1"][:], rhs, start=True, stop=True)
                    src = ps1[:].rearrange("p (r c) -> p r c", c=64)
                    dst_t = _rview(t["pad1"][0:64, :], base + 2, nrows)
                    nc.vector.tensor_scalar(
                        dst_t, src, t["b1"][:, 0:1], 0.0, op0=ADD, op1=MAX
                    )
                    dst_b = _rview(t["pad1"][64:128, :], base + 2 - PW, nrows)
                    nc.vector.tensor_scalar(
                        dst_b, src, t["b1"][:, 0:1], 0.0, op0=ADD, op1=MAX
                    )

            def emit_conv2(t):
                # 6 K=128 matmuls per chunk (3 dy-paired + 3 single);
                # relu+bias drain on DVE
                for r0, nrows in RCHUNKS:
                    n = nrows * 64
                    base = r0 * PW
                    ps2 = psum.tile([128, n], F32, name="ps2", tag="ps2")
                    for j, dd in enumerate(C2_DELTAS):
                        lhsT = t["w2"][:, j * 128 : (j + 1) * 128]
                        rhs = _rview(t["pad1"], base + 2 + dd, nrows)
                        nc.tensor.matmul(
                            ps2[:], lhsT, rhs,
                            start=(j == 0), stop=(j == len(C2_DELTAS) - 1),
                        )
                    src = ps2[:].rearrange("p (r c) -> p r c", c=64)
                    dst = _rview(t["pad2"], base + 2, nrows)
                    nc.vector.tensor_scalar(
                        dst, src, t["b2"][:, 0:1], 0.0, op0=ADD, op1=MAX
                    )

            def emit_conv3_half(t, h):
                # 9 shifts; relu+bias drain on ACT with accum_out -> pooling
                for k, (r0, nrows) in enumerate(RCHUNKS):
                    n = nrows * 64
                    base = r0 * PW
                    ps3 = psum.tile([128, n], F32, name="ps3", tag="ps3")
                    for i, dd in enumerate(DELTAS):
                        lhsT = t["w3"][:, i * 256 + h * 128 : i * 256 + h * 128 + 128]
                        rhs = _rview(t["pad2"], base + 2 + dd, nrows)
                        nc.tensor.matmul(
                            ps3[:], lhsT, rhs, start=(i == 0), stop=(i == 8)
                        )
                    scr = scrpool.tile([128, n], F32, name="scr", tag="scr")
                    idx = h * 8 + k
                    nc.scalar.activation(
                        scr[:], ps3[:], RELU,
                        bias=t["b3"][:, h : h + 1],
                        accum_out=t["acc"][:, idx : idx + 1],
                    )

            def emit_fc(s, t):
                pooled = spool.tile([128, 2], F32, name="pooled", tag="pooled")
                nc.vector.tensor_reduce(
                    pooled[:],
                    t["acc"][:].rearrange("p (h o) -> p h o", h=2),
                    axis=mybir.AxisListType.X,
                    op=ADD,
                )
                outsb = spool.tile([128, 20], F32, name="outsb", tag="outsb")
                for h in range(2):
                    tmp = spool.tile([128, 10], F32, name="tmp", tag="tmp")
                    nc.vector.tensor_scalar_mul(
                        tmp[:], t["fcb"][:, 0:10], pooled[:, h : h + 1]
                    )
                    nc.vector.tensor_add(
                        outsb[:, h * 10 : h * 10 + 10], tmp[:], t["fcb"][:, 10:20]
                    )
                nc.sync.dma_start(out_d[s], outsb[:])

            # software-pipelined emission: conv1 of s+1 sits between the two
            # conv3 halves of s, so the PE stream never starves
            T[0] = emit_loads(0)
            emit_conv1(T[0])
            for s in range(SPC):
                t = T[s]
                t["acc"] = spool.tile([128, 16], F32, name="acc", tag="acc")
                emit_conv2(t)
                emit_conv3_half(t, 0)
                if s + 1 < SPC:
                    T[s + 1] = emit_loads(s + 1)
                    emit_conv1(T[s + 1])
                emit_conv3_half(t, 1)
                emit_fc(s, t)
                T[s] = None
    nc.compile()
    return nc


def prep_inputs(x, conv1_weight, conv2_weight, conv3_weight, fc_weight,
                bias1, bias2, bias3, bias4):
    """Host-side layout prep (pure data movement, no model math)."""
    import ml_dtypes

    f = np.float32
    bf = ml_dtypes.bfloat16
    x = np.asarray(x, f)
    padx = np.zeros((B, 3, PH, PW), f)
    padx[:, :, 1:65, 1:65] = x
    padflat = padx.reshape(B, 3, NPIX)
    xim = np.zeros((B, 27, NPIX), f)
    for s, d in enumerate(DELTAS):
        lo = max(0, -d)
        hi = min(NPIX, NPIX - d)
        xim[:, s * 3 : s * 3 + 3, lo:hi] = padflat[:, :, lo + d : hi + d]

    w1 = np.ascontiguousarray(
        np.asarray(conv1_weight, f).transpose(0, 3, 4, 2, 1).reshape(B, 27, 64)
    )
    # conv2 stacked-pair weights: [b, 2*ci, 6, co]
    w2n = np.asarray(conv2_weight, f).transpose(0, 2, 3, 4, 1).reshape(B, 64, 9, 128)
    w2p = np.zeros((B, 128, 6, 128), f)
    for j in range(3):
        w2p[:, 0:64, j] = w2n[:, :, j]          # dy=0, dx=j  (top half)
        w2p[:, 64:128, j] = w2n[:, :, 3 + j]    # dy=1, dx=j  (shifted bottom)
        w2p[:, 0:64, 3 + j] = w2n[:, :, 6 + j]  # dy=2, dx=j  (top, bottom=0)
    w2 = np.ascontiguousarray(w2p.reshape(B, 128, 6 * 128))
    w3 = np.ascontiguousarray(
        np.asarray(conv3_weight, f).transpose(0, 2, 3, 4, 1).reshape(B, 128, 9 * 256)
    )
    b1 = np.ascontiguousarray(np.asarray(bias1, f)[:, :, None])
    b2 = np.ascontiguousarray(np.asarray(bias2, f)[:, :, None])
    b3 = np.ascontiguousarray(np.asarray(bias3, f).reshape(B, 2, 128).transpose(0, 2, 1))
    fcs = np.asarray(fc_weight, f)[:, 0, :] / np.float32(H * W)
    fcb = np.concatenate(
        [
            np.repeat(fcs[:, None, :], 128, axis=1),
            np.repeat(np.asarray(bias4, f)[:, None, :], 128, axis=1),
        ],
        axis=2,
    )
    fcb = np.ascontiguousarray(fcb)
    return (xim.astype(bf), w1.astype(bf), w2.astype(bf), w3.astype(bf),
            b1, b2, b3, fcb)


_NC_CACHE = {}
LAST_RESULTS = None


def kernel(x, conv1_weight, conv2_weight, conv3_weight, fc_weight,
           bias1, bias2, bias3, bias4):
    global LAST_RESULTS
    xim, w1, w2, w3, b1, b2, b3, fcb = prep_inputs(
        x, conv1_weight, conv2_weight, conv3_weight, fc_weight,
        bias1, bias2, bias3, bias4,
    )
    if "nc" not in _NC_CACHE:
        _NC_CACHE["nc"] = build_nc()
    nc = _NC_CACHE["nc"]

    in_maps = []
    for c in range(N_CORES):
        sl = slice(c * SPC, (c + 1) * SPC)
        in_maps.append(
            {
                "xim": np.ascontiguousarray(xim[sl]),
                "w1": np.ascontiguousarray(w1[sl]),
                "w2": np.ascontiguousarray(w2[sl]),
                "w3": np.ascontiguousarray(w3[sl]),
                "b1": np.ascontiguousarray(b1[sl]),
                "b2": np.ascontiguousarray(b2[sl]),
                "b3": np.ascontiguousarray(b3[sl]),
                "fcb": np.ascontiguousarray(fcb[sl]),
            }
        )
    res = run_bass_kernel_spmd(nc, in_maps, list(range(N_CORES)))
    LAST_RESULTS = res
    outs = []
    for c in range(N_CORES):
        o = np.asarray(res.results[c]["out"], np.float32)  # [SPC, 128, 20]
        outs.append(o.reshape(SPC, 128, 2, 10).transpose(0, 2, 1, 3).reshape(SPC, 256, 10))
    return np.concatenate(outs, axis=0)


# revision 12
# speedup vs baseline: 1.5627x; 1.0468x over previous
"""Trainium2 Bass kernel for ConvNetWithGlobalPooling (batch-parallel grouped CNN).

Per-sample network: 3x(3x3 SAME conv + per-sample bias + relu) ->
global mean pool -> per-sample outer product with fc vector + bias.

Sharding: pure data parallel, 4 samples per core across 8 cores.

Device strategy (per sample):
  - Activations live in SBUF in "padded flat" layout: a [C, 66*66] image with
    zero borders stored in a [C, 4360] buffer (image at buffer[1 : 4357]).
  - A 3x3 SAME conv output at padded-flat position i is
        sum_s W[s] @ x_pad[i + delta_s],  delta_s = (dy-1)*66 + (dx-1)
    so each shift's rhs is a strided view of the padded buffer and shifts
    accumulate into one PSUM bank (K-tiled im2col with zero data movement).
    Matmul rhs uses a 2-level AP ([rows x 66-stride][64 cols]) so only the 64
    valid columns of each row are streamed (no garbage columns computed).
  - conv1 (Cin=3) uses a host-built im2col (27 rows = 9 shifts x 3 chans):
    a single K=27 matmul per chunk.
  - conv2 shift-pairing: conv1's relu-drain writes its output TWICE into a
    [128, 4360] buffer - rows 0:64 normal, rows 64:128 shifted down one image
    row (+66). A K=128 matmul against stacked weights then computes TWO
    shifts at once: dy=0 paired with dy=1 (3 matmuls), dy=2 alone in the top
    half with zero bottom weights (3 matmuls) -> 6 streams instead of 9.
  - Output rows are processed in 8 chunks of 8 rows (N=512 = one PSUM bank).
  - All matmul operands are bf16 (1 PE cycle/row vs 2 for fp32/fp32r),
    fp32 PSUM accumulation.
  - Engine split: PE does matmuls; DVE does conv1/conv2 relu+bias drains
    (tensor_scalar add-bias/max-0) and the fc stage; ACT does conv3 drains
    because activation's accum_out gives the global-sum pooling for free
    (1/4096 folded into the fc weight on host).
  - Emission is software-pipelined: conv1 of sample s+1 is emitted between
    conv3's two Cout halves of sample s so the PE never starves at sample
    boundaries (which would also re-engage the HAM half-clock throttle).
"""

import os
import sys

sys.path.insert(0, "/opt/trn_rl_repo")

import numpy as np

import concourse.bass as bass
import concourse.bacc as bacc
import concourse.tile as tile
from concourse import mybir
from concourse.bass_utils import run_bass_kernel_spmd

F32 = mybir.dt.float32
BF16 = mybir.dt.bfloat16
RELU = mybir.ActivationFunctionType.Relu
ADD = mybir.AluOpType.add
MAX = mybir.AluOpType.max

B = 32
N_CORES = 8
SPC = B // N_CORES  # samples per core
H = W = 64
PW = W + 2  # 66
PH = H + 2
NPIX = PH * PW  # 4356
PADLEN = NPIX + 4  # 1 slack front + 3 slack tail; image at [1, 4357)
ROWS_PER_CHUNK = 8  # N = 8*64 = 512 = exactly one PSUM bank of fp32
RCHUNKS = [(1 + 8 * k, 8) for k in range(8)]
DELTAS = [(dy - 1) * PW + (dx - 1) for dy in range(3) for dx in range(3)]
# conv2 pairing: matmul j<3 pairs (dy=0,dx=j) with (dy=1,dx=j) via the
# row-shifted bottom half; j=3..5 is (dy=2,dx=j-3) in the top half only.
C2_DELTAS = [DELTAS[j] for j in range(3)] + [DELTAS[6 + j] for j in range(3)]


def _rview(ap, off, nrows):
    """[P, nrows, 64] view of a padded-flat buffer starting at `off`."""
    return ap[:, off : off + nrows * PW].rearrange("p (r c) -> p r c", c=PW)[
        :, :, 0:64
    ]


def build_nc():
    nc = bacc.Bacc(
        "TRN2",
        target_bir_lowering=False,
        debug=False,
        num_devices=N_CORES,
    )
    xim_d = nc.declare_dram_parameter("xim", [SPC, 27, NPIX], BF16, isOutput=False)
    w1_d = nc.declare_dram_parameter("w1", [SPC, 27, 64], BF16, isOutput=False)
    w2_d = nc.declare_dram_parameter("w2", [SPC, 128, 6 * 128], BF16, isOutput=False)
    w3_d = nc.declare_dram_parameter("w3", [SPC, 128, 9 * 256], BF16, isOutput=False)
    b1_d = nc.declare_dram_parameter("b1", [SPC, 64, 1], F32, isOutput=False)
    b2_d = nc.declare_dram_parameter("b2", [SPC, 128, 1], F32, isOutput=False)
    b3_d = nc.declare_dram_parameter("b3", [SPC, 128, 2], F32, isOutput=False)
    fcb_d = nc.declare_dram_parameter("fcb", [SPC, 128, 20], F32, isOutput=False)
    out_d = nc.declare_dram_parameter("out", [SPC, 128, 20], F32, isOutput=True)

    with tile.TileContext(nc) as tc:
        with (
            tc.tile_pool(name="wpool", bufs=2) as wpool,
            tc.tile_pool(name="apool", bufs=2) as apool,
            tc.tile_pool(name="spool", bufs=2) as spool,
            tc.tile_pool(name="scrpool", bufs=4) as scrpool,
            tc.tile_pool(name="psum", bufs=2, space="PSUM") as psum,
        ):
            T = [None] * SPC  # per-sample tile dict

            def emit_loads(s):
                t = {}
                t["w1"] = wpool.tile([27, 64], BF16, name="w1", tag="w1")
                nc.sync.dma_start(t["w1"][:], w1_d[s])
                t["b1"] = spool.tile([64, 1], F32, name="b1", tag="b1")
                nc.sync.dma_start(t["b1"][:], b1_d[s])
                t["xim"] = apool.tile([27, NPIX], BF16, name="xim", tag="xim")
                # small head piece (first 2 chunks) so conv1 starts early,
                # then the bulk in one efficient DMA
                nc.sync.dma_start(t["xim"][:, 67:1123], xim_d[s][:, 67:1123])
                nc.sync.dma_start(t["xim"][:, 1123:4291], xim_d[s][:, 1123:4291])
                t["w2"] = wpool.tile([128, 6 * 128], BF16, name="w2", tag="w2")
                nc.sync.dma_start(t["w2"][:], w2_d[s])
                t["w3"] = wpool.tile([128, 9 * 256], BF16, name="w3", tag="w3")
                nc.sync.dma_start(t["w3"][:], w3_d[s])
                t["b2"] = spool.tile([128, 1], F32, name="b2", tag="b2")
                nc.sync.dma_start(t["b2"][:], b2_d[s])
                t["b3"] = spool.tile([128, 2], F32, name="b3", tag="b3")
                nc.sync.dma_start(t["b3"][:], b3_d[s])
                t["fcb"] = spool.tile([128, 20], F32, name="fcb", tag="fcb")
                nc.sync.dma_start(t["fcb"][:], fcb_d[s])
                # pad1: rows 0:64 = conv1 out (padded), rows 64:128 = same
                # image shifted down one row (content[i] = top[i+66])
                t["pad1"] = apool.tile([128, PADLEN], BF16, name="pad1", tag="pad1")
                nc.vector.memset(t["pad1"][:], 0.0)
                t["pad2"] = apool.tile([128, PADLEN], BF16, name="pad2", tag="pad2")
                nc.vector.memset(t["pad2"][:], 0.0)
                return t

            def emit_conv1(t):
                # K=27 matmul per chunk; relu+bias drains on DVE into both
                # halves of pad1
                for r0, nrows in RCHUNKS:
                    n = nrows * 64
                    base = r0 * PW
                    ps1 = psum.tile([64, n], F32, name="ps1", tag="ps1")
                    rhs = _rview(t["xim"], base + 1, nrows)
                    nc.tensor.matmul(ps1[:], t["w1"][:], rhs, start=True, stop=True)
                    src = ps1[:].rearrange("p (r c) -> p r c", c=64)
                    dst_t = _rview(t["pad1"][0:64, :], base + 2, nrows)
                    nc.scalar.activation(dst_t, src, RELU, bias=t["b1"][:, 0:1])
                    dst_b = _rview(t["pad1"][64:128, :], base + 2 - PW, nrows)
                    nc.vector.tensor_scalar(
                        dst_b, src, t["b1"][:, 0:1], 0.0, op0=ADD, op1=MAX
                    )

            def emit_conv2(t):
                # 6 K=128 matmuls per chunk (3 dy-paired + 3 single);
                # relu+bias drain on DVE
                for r0, nrows in RCHUNKS:
                    n = nrows * 64
                    base = r0 * PW
                    ps2 = psum.tile([128, n], F32, name="ps2", tag="ps2")
                    for j, dd in enumerate(C2_DELTAS):
                        lhsT = t["w2"][:, j * 128 : (j + 1) * 128]
                        rhs = _rview(t["pad1"], base + 2 + dd, nrows)
                        nc.tensor.matmul(
                            ps2[:], lhsT, rhs,
                            start=(j == 0), stop=(j == len(C2_DELTAS) - 1),
                        )
                    src = ps2[:].rearrange("p (r c) -> p r c", c=64)
                    dst = _rview(t["pad2"], base + 2, nrows)
                    nc.vector.tensor_scalar(
                        dst, src, t["b2"][:, 0:1], 0.0, op0=ADD, op1=MAX
                    )

            def emit_conv3_half(t, h):
                # 9 shifts; relu+bias drain on ACT with accum_out -> pooling
                for k, (r0, nrows) in enumerate(RCHUNKS):
                    n = nrows * 64
                    base = r0 * PW
                    ps3 = psum.tile([128, n], F32, name="ps3", tag="ps3", bufs=3)
                    for i, dd in enumerate(DELTAS):
                        lhsT = t["w3"][:, i * 256 + h * 128 : i * 256 + h * 128 + 128]
                        rhs = _rview(t["pad2"], base + 2 + dd, nrows)
                        nc.tensor.matmul(
                            ps3[:], lhsT, rhs, start=(i == 0), stop=(i == 8)
                        )
                    scr = scrpool.tile([128, n], F32, name="scr", tag="scr")
                    idx = h * 8 + k
                    nc.scalar.activation(
                        scr[:], ps3[:], RELU,
                        bias=t["b3"][:, h : h + 1],
                        accum_out=t["acc"][:, idx : idx + 1],
                    )

            def emit_fc(s, t):
                pooled = spool.tile([128, 2], F32, name="pooled", tag="pooled")
                nc.vector.tensor_reduce(
                    pooled[:],
                    t["acc"][:].rearrange("p (h o) -> p h o", h=2),
                    axis=mybir.AxisListType.X,
                    op=ADD,
                )
                outsb = spool.tile([128, 20], F32, name="outsb", tag="outsb")
                for h in range(2):
                    tmp = spool.tile([128, 10], F32, name="tmp", tag="tmp")
                    nc.vector.tensor_scalar_mul(
                        tmp[:], t["fcb"][:, 0:10], pooled[:, h : h + 1]
                    )
                    nc.vector.tensor_add(
                        outsb[:, h * 10 : h * 10 + 10], tmp[:], t["fcb"][:, 10:20]
                    )
                nc.sync.dma_start(out_d[s], outsb[:])

            # software-pipelined emission: conv1 of s+1 sits between the two
            # conv3 halves of s, so the PE stream never starves
            T[0] = emit_loads(0)
            emit_conv1(T[0])
            for s in range(SPC):
                t = T[s]
                t["acc"] = spool.tile([128, 16], F32, name="acc", tag="acc")
                emit_conv2(t)
                emit_conv3_half(t, 0)
                if s + 1 < SPC:
                    T[s + 1] = emit_loads(s + 1)
                    emit_conv1(T[s + 1])
                emit_conv3_half(t, 1)
                emit_fc(s, t)
                T[s] = None
    nc.compile()
    return nc


def prep_inputs(x, conv1_weight, conv2_weight, conv3_weight, fc_weight,
                bias1, bias2, bias3, bias4):
    """Host-side layout prep (pure data movement, no model math)."""
    import ml_dtypes

    f = np.float32
    bf = ml_dtypes.bfloat16
    x = np.asarray(x, f)
    padx = np.zeros((B, 3, PH, PW), f)
    padx[:, :, 1:65, 1:65] = x
    padflat = padx.reshape(B, 3, NPIX)
    xim = np.zeros((B, 27, NPIX), f)
    for s, d in enumerate(DELTAS):
        lo = max(0, -d)
        hi = min(NPIX, NPIX - d)
        xim[:, s * 3 : s * 3 + 3, lo:hi] = padflat[:, :, lo + d : hi + d]

    w1 = np.ascontiguousarray(
        np.asarray(conv1_weight, f).transpose(0, 3, 4, 2, 1).reshape(B, 27, 64)
    )
    # conv2 stacked-pair weights: [b, 2*ci, 6, co]
    w2n = np.asarray(conv2_weight, f).transpose(0, 2, 3, 4, 1).reshape(B, 64, 9, 128)
    w2p = np.zeros((B, 128, 6, 128), f)
    for j in range(3):
        w2p[:, 0:64, j] = w2n[:, :, j]          # dy=0, dx=j  (top half)
        w2p[:, 64:128, j] = w2n[:, :, 3 + j]    # dy=1, dx=j  (shifted bottom)
        w2p[:, 0:64, 3 + j] = w2n[:, :, 6 + j]  # dy=2, dx=j  (top, bottom=0)
    w2 = np.ascontiguousarray(w2p.reshape(B, 128, 6 * 128))
    w3 = np.ascontiguousarray(
        np.asarray(conv3_weight, f).transpose(0, 2, 3, 4, 1).reshape(B, 128, 9 * 256)
    )
    b1 = np.ascontiguousarray(np.asarray(bias1, f)[:, :, None])
    b2 = np.ascontiguousarray(np.asarray(bias2, f)[:, :, None])
    b3 = np.ascontiguousarray(np.asarray(bias3, f).reshape(B, 2, 128).transpose(0, 2, 1))
    fcs = np.asarray(fc_weight, f)[:, 0, :] / np.float32(H * W)
    fcb = np.concatenate(
        [
            np.repeat(fcs[:, None, :], 128, axis=1),
            np.repeat(np.asarray(bias4, f)[:, None, :], 128, axis=1),
        ],
        axis=2,
    )
    fcb = np.ascontiguousarray(fcb)
    return (xim.astype(bf), w1.astype(bf), w2.astype(bf), w3.astype(bf),
            b1, b2, b3, fcb)


_NC_CACHE = {}
LAST_RESULTS = None


def kernel(x, conv1_weight, conv2_weight, conv3_weight, fc_weight,
           bias1, bias2, bias3, bias4):
    global LAST_RESULTS
    xim, w1, w2, w3, b1, b2, b3, fcb = prep_inputs(
        x, conv1_weight, conv2_weight, conv3_weight, fc_weight,
        bias1, bias2, bias3, bias4,
    )
    if "nc" not in _NC_CACHE:
        _NC_CACHE["nc"] = build_nc()
    nc = _NC_CACHE["nc"]

    in_maps = []
    for c in range(N_CORES):
        sl = slice(c * SPC, (c + 1) * SPC)
        in_maps.append(
            {
                "xim": np.ascontiguousarray(xim[sl]),
                "w1": np.ascontiguousarray(w1[sl]),
                "w2": np.ascontiguousarray(w2[sl]),
                "w3": np.ascontiguousarray(w3[sl]),
                "b1": np.ascontiguousarray(b1[sl]),
                "b2": np.ascontiguousarray(b2[sl]),
                "b3": np.ascontiguousarray(b3[sl]),
                "fcb": np.ascontiguousarray(fcb[sl]),
            }
        )
    res = run_bass_kernel_spmd(nc, in_maps, list(range(N_CORES)))
    LAST_RESULTS = res
    outs = []
    for c in range(N_CORES):
        o = np.asarray(res.results[c]["out"], np.float32)  # [SPC, 128, 20]
        outs.append(o.reshape(SPC, 128, 2, 10).transpose(0, 2, 1, 3).reshape(SPC, 256, 10))
    return np.concatenate(outs, axis=0)
